# revision 1
# baseline (speedup 1.0000x reference)
"""Trainium2 Bass kernel for cached multi-head self-attention decode step.

Problem (hardcoded):
  B=16, T=8, C=1024, n_head=16, head_dim=64, Lcache=4096, layer index 1.
  reference:
    q = x@Wq.T + bq ; key = x@Wk.T ; value = x@Wv.T + bv
    K = concat(kv_cache[:,1,0], key) ; V = concat(kv_cache[:,1,1], value)
    out = softmax((q*s)(K*s)^T) @ V @ Wo.T + bo      (s = hd**-0.25)
    returns (out, key, value)

Sharding: data-parallel over batch. 8 cores x 2 batches each. No collectives.

Per-core device algorithm (per batch b):
  - projections: q/k/v = xT.T @ [WqT|WkT|WvT]  (xT stationary, fp32)
  - PE-transpose q,k -> qT,kT [c, m] layouts
  - QbigT[c-chunk, (h,m)]: block-diagonal packing of qT so that ONE
    [128 x L] matmul chain computes all 16 heads x 8 queries of scores:
      S[(h,m), l] = sum_c QbigT[c,(h,m)] * KT[c, l]
  - softmax without max-subtraction (scores ~ N(0,1), no overflow):
      exp fused into the PSUM->SBUF copy, row-sums via activation accum_out,
      normalization deferred to the attention output (cheaper).
  - S@V: W-chunks PE-transposed ([l,128] lhsT), V streamed in natural
    layout, PSUM-accumulated over 33 l-chunks (last chunk = new kv rows,
    zero-padded to 128).
  - out = (gathered wv)^T chunks @ WoT + bo.

The K-cache is transposed on the HOST (numpy) so every device DMA is a
contiguous full-bandwidth load. Attention-core tensors (KT/V/q/k/W) can
optionally run in bf16 (halves the dominant KV DMA); projections and the
key/value outputs always stay fp32.
"""

import sys
import types

import numpy as np
import ml_dtypes

# ---- hardcoded problem geometry ----
B, T, C = 16, 8, 1024
H, HD = 16, 64
L = 4096            # cached length
LT = L + T          # total keys
NCORES = 8
BPC = B // NCORES   # batches per core = 2
M = BPC * T         # queries per core = 16
P = 128
CH = C // P         # 8 c-chunks
NW = L // 512       # 8 score windows of 512
NV = 4              # V l-chunks per DMA
SCALE = float(HD) ** -0.5  # applied once on q

# attention-core dtype: "bf16" halves KV DMA (the roofline term); fp32 exact.
ATTN_DT = "bf16"

_CACHE = {}


def _ensure_ntff_hook():
    """run_bass_kernel_spmd(trace=True) under axon needs antenv.axon_hooks;
    shim it from the boot module if the image's antenv lacks it."""
    try:
        import antenv.axon_hooks  # noqa: F401
        return
    except ImportError:
        pass
    try:
        import trn_agent_boot.trn_boot as tb
        hook = tb._ntff_profile_via_ctypes("/opt/axon/libaxon_pjrt.so")
    except Exception:
        hook = None
    mod = types.ModuleType("antenv.axon_hooks")
    mod.get_axon_ntff_profile_hook = lambda: hook
    mod.set_axon_ntff_profile_hook = lambda h: None
    sys.modules["antenv.axon_hooks"] = mod


def _build(attn_dt_name: str):
    import concourse.bacc as bacc
    import concourse.mybir as mybir
    import concourse.tile as tile
    from concourse.masks import make_identity

    f32 = mybir.dt.float32
    adt = {"bf16": mybir.dt.bfloat16, "f32": f32}[attn_dt_name]

    nc = bacc.Bacc(None, target_bir_lowering=False)

    # ---- dram I/O ----
    xTr = nc.dram_tensor("xTr", [P, CH, M], f32, kind="ExternalInput")
    KT = nc.dram_tensor("KT", [BPC, CH, P, L], adt, kind="ExternalInput")
    Vd = nc.dram_tensor("Vd", [BPC, L // P, P, C], adt, kind="ExternalInput")
    Wqkv = nc.dram_tensor("Wqkv", [CH, P, 3 * C], f32, kind="ExternalInput")
    WoT = nc.dram_tensor("WoT", [CH, P, C], f32, kind="ExternalInput")
    bqs = nc.dram_tensor("bqs", [P, CH], f32, kind="ExternalInput")
    bvb = nc.dram_tensor("bvb", [M, C], f32, kind="ExternalInput")
    bob = nc.dram_tensor("bob", [M, C], f32, kind="ExternalInput")
    out_d = nc.dram_tensor("out", [M, C], f32, kind="ExternalOutput")
    key_d = nc.dram_tensor("key", [M, C], f32, kind="ExternalOutput")
    val_d = nc.dram_tensor("value", [M, C], f32, kind="ExternalOutput")

    AF = mybir.ActivationFunctionType
    AX = mybir.AxisListType
    OP = mybir.AluOpType

    with tile.TileContext(nc) as tc:
        with (
            tc.tile_pool(name="const", bufs=1) as cpool,
            tc.tile_pool(name="weights", bufs=3) as wpool,
            tc.tile_pool(name="kt", bufs=2 if attn_dt_name == "f32" else 4) as ktpool,
            tc.tile_pool(name="v", bufs=2 if attn_dt_name == "f32" else 6) as vpool,
            tc.tile_pool(name="wchunk", bufs=6) as wtpool,
            tc.tile_pool(name="big", bufs=1) as big,
            tc.tile_pool(name="ps", bufs=1, space="PSUM") as pp,
        ):
            # ---- constants ----
            ident = cpool.tile([P, P], f32, tag="ident", name="ident")
            make_identity(nc, ident)
            if adt != f32:
                ident_a = cpool.tile([P, P], adt, tag="ident_a", name="ident_a")
                nc.vector.tensor_copy(out=ident_a[:], in_=ident[:])
            else:
                ident_a = ident
            xT_sb = cpool.tile([P, CH, M], f32, tag="xT", name="xT")
            nc.sync.dma_start(xT_sb[:], xTr[:])
            bqs_sb = cpool.tile([P, CH], f32, tag="bqs", name="bqs")
            nc.sync.dma_start(bqs_sb[:], bqs[:])
            bvb_sb = cpool.tile([M, C], f32, tag="bvb", name="bvb")
            nc.sync.dma_start(bvb_sb[:], bvb[:])
            bob_sb = cpool.tile([M, C], f32, tag="bob", name="bob")
            nc.sync.dma_start(bob_sb[:], bob[:])

            # ---- stage A: projections q/k/v (fp32) ----
            # psum tags: 8 banks total, reused across stages.
            ps_proj = [pp.tile([M, 512], f32, tag=t, name=f"proj_{t}")
                       for t in ("o0", "o1", "o2", "o3", "s0", "s1")]
            for ci in range(CH):
                wt = wpool.tile([P, 3 * C], f32, tag="wqkv", name="wqkv")
                nc.scalar.dma_start(wt[:], Wqkv[ci])
                for j, ps in enumerate(ps_proj):
                    nc.tensor.matmul(
                        ps[:], xT_sb[:, ci, :], wt[:, j * 512:(j + 1) * 512],
                        start=(ci == 0), stop=(ci == CH - 1),
                    )

            q_nat = big.tile([P, C], f32, tag="q_nat", name="q_nat")
            k_nat = big.tile([P, C], f32, tag="k_nat", name="k_nat")
            v_nat = big.tile([P, C], f32, tag="v_nat", name="v_nat")
            for t in (q_nat, k_nat, v_nat):
                nc.gpsimd.memset(t[:], 0.0)
            for j in range(2):
                sl = slice(j * 512, (j + 1) * 512)
                nc.scalar.copy(q_nat[0:M, sl], ps_proj[0 + j][:])
                nc.scalar.copy(k_nat[0:M, sl], ps_proj[2 + j][:])
                nc.scalar.copy(v_nat[0:M, sl], ps_proj[4 + j][:])
            nc.vector.tensor_add(out=v_nat[0:M, :], in0=v_nat[0:M, :], in1=bvb_sb[:])
            nc.sync.dma_start(key_d[:], k_nat[0:M, :])
            nc.sync.dma_start(val_d[:], v_nat[0:M, :])

            # ---- qT / kT via PE transpose ----
            qT = big.tile([P, CH, M], adt, tag="qT", name="qT")
            kT = big.tile([P, CH, M], adt, tag="kT", name="kT")
            for ci in range(CH):
                sl = slice(ci * P, (ci + 1) * P)
                tp = pp.tile([P, P], f32, tag="t0", name="t0")
                nc.tensor.transpose(tp[:], q_nat[:, sl], ident)
                # qT = (q + bq) * scale  (bqs host-prescaled by SCALE)
                nc.scalar.activation(
                    qT[:, ci, :], tp[:, 0:M], AF.Identity,
                    bias=bqs_sb[:, ci:ci + 1], scale=SCALE,
                )
                tp2 = pp.tile([P, P], f32, tag="t1", name="t1")
                nc.tensor.transpose(tp2[:], k_nat[:, sl], ident)
                nc.vector.tensor_copy(out=kT[:, ci, :], in_=tp2[:, 0:M])

            # ---- attention: both batches interleaved for PE/DMA overlap ----
            Qb, W_s, sums, rsum, ops_b, On = {}, {}, {}, {}, {}, {}
            for b in range(BPC):
                # QbigT: block-diagonal packing [c-chunk, (h, m)]
                Qb[b] = big.tile([P, CH, P], adt, tag=f"Qbig{b}", name=f"Qbig{b}")
                nc.gpsimd.memset(Qb[b][:], 0.0)
                for ci in range(CH):
                    for j in range(2):
                        rows = slice(64 * j, 64 * (j + 1))
                        col0 = 16 * ci + 8 * j
                        nc.vector.tensor_copy(
                            out=Qb[b][rows, ci, col0:col0 + 8],
                            in_=qT[rows, ci, b * T:(b + 1) * T],
                        )
                W_s[b] = big.tile([P, LT], adt, tag=f"W{b}", name=f"W{b}")
                sums[b] = big.tile([P, 16], f32, tag=f"sums{b}", name=f"sums{b}")

            # scores + exp (softmax numerator); row-sums via accum_out
            for lw in range(NW):
                for b in range(BPC):
                    kt = ktpool.tile([P, CH, 512], adt, tag="kt", name="kt")
                    nc.sync.dma_start(
                        kt[:],
                        KT[b].rearrange("ch pi l -> pi ch l")[
                            :, :, lw * 512:(lw + 1) * 512],
                    )
                    sp = pp.tile([P, 512], f32, tag=f"s{(2 * lw + b) % 2}",
                                 name="sp")
                    for ci in range(CH):
                        nc.tensor.matmul(
                            sp[:], Qb[b][:, ci, :], kt[:, ci, :],
                            start=(ci == 0), stop=(ci == CH - 1),
                        )
                    nc.scalar.activation(
                        W_s[b][:, lw * 512:(lw + 1) * 512], sp[:], AF.Exp,
                        accum_out=sums[b][:, lw:lw + 1],
                    )
            for b in range(BPC):
                # scores against the T new keys
                spn = pp.tile([P, 512], f32, tag=f"s{b % 2}", name="spn")
                for ci in range(CH):
                    nc.tensor.matmul(
                        spn[:, 0:T], Qb[b][:, ci, :], kT[:, ci, b * T:(b + 1) * T],
                        start=(ci == 0), stop=(ci == CH - 1),
                    )
                nc.scalar.activation(
                    W_s[b][:, L:LT], spn[:, 0:T], AF.Exp,
                    accum_out=sums[b][:, NW:NW + 1],
                )
                rs = big.tile([P, 1], f32, tag=f"rs{b}", name=f"rs{b}")
                nc.vector.tensor_reduce(
                    out=rs[:], in_=sums[b][:, 0:NW + 1], axis=AX.X, op=OP.add)
                rsum[b] = big.tile([P, 1], f32, tag=f"rsum{b}", name=f"rsum{b}")
                nc.vector.reciprocal(rsum[b][:], rs[:])

                # padded last-chunk operands (new kv rows). vpad rows 0:16
                # hold BOTH batches' new v rows; each batch's W^T_new lands on
                # its own row range via a free-dim shifted transpose input, so
                # every partition start stays 32-aligned.
                if b == 0:
                    vpad = big.tile([P, C], adt, tag="vpad", name="vpad")
                    nc.gpsimd.memset(vpad[:], 0.0)
                    nc.vector.tensor_copy(out=vpad[0:M, :], in_=v_nat[0:M, :])
                wt32 = big.tile([P, P], adt, tag=f"wt32_{b}", name=f"wt32_{b}")
                nc.gpsimd.memset(wt32[:], 0.0)
                wn_pad = big.tile([P, M], adt, tag=f"wn_pad{b}", name=f"wn_pad{b}")
                nc.gpsimd.memset(wn_pad[:], 0.0)
                nc.vector.tensor_copy(
                    out=wn_pad[:, b * T:(b + 1) * T], in_=W_s[b][:, L:LT])
                tpn = pp.tile([P, P], adt, tag="t0", name="tpn")
                nc.tensor.transpose(tpn[0:M, :], wn_pad[:], ident_a)
                nc.vector.tensor_copy(out=wt32[0:M, :], in_=tpn[0:M, :])
                ops_b[b] = (wt32,
                            [pp.tile([P, 512], f32, tag=f"o{2 * b + j}",
                                     name=f"o{2 * b + j}") for j in range(2)])

            # S@V accumulated over 33 l-chunks, batches interleaved.
            # V loads ride the second HWDGE FIFO (scalar) so they are not
            # queued behind the KT stream.
            nt = L // P  # 32 cached chunks
            for tsup in range(nt // NV):
                for b in range(BPC):
                    ops = ops_b[b][1]
                    vt = vpool.tile([P, NV, C], adt, tag="v", name="v")
                    nc.scalar.dma_start(
                        vt[:],
                        Vd[b].rearrange("t pi c -> pi t c")[
                            :, tsup * NV:(tsup + 1) * NV, :],
                    )
                    for tt in range(NV):
                        t_ = tsup * NV + tt
                        tpw = pp.tile([P, P], adt, tag=f"t{(2 * t_ + b) % 2}",
                                      name="tpw")
                        nc.tensor.transpose(
                            tpw[:], W_s[b][:, t_ * P:(t_ + 1) * P], ident_a)
                        wts = wtpool.tile([P, P], adt, tag="wt", name="wt")
                        nc.vector.tensor_copy(out=wts[:], in_=tpw[:])
                        for j in range(2):
                            nc.tensor.matmul(
                                ops[j][:], wts[:], vt[:, tt, j * 512:(j + 1) * 512],
                                start=(t_ == 0), stop=False,
                            )
            for b in range(BPC):
                wt32, ops = ops_b[b]
                for j in range(2):
                    nc.tensor.matmul(
                        ops[j][:], wt32[:], vpad[:, j * 512:(j + 1) * 512],
                        start=False, stop=True,
                    )
                # normalize rows while copying out of PSUM
                On[b] = big.tile([P, C], f32, tag=f"On{b}", name=f"On{b}")
                for j in range(2):
                    nc.scalar.activation(
                        On[b][:, j * 512:(j + 1) * 512], ops[j][:], AF.Copy,
                        scale=rsum[b][:],
                    )

            # ---- wvT directly from transposed On chunks ----
            # OnT[c_local, (h,t)] = On[8h+t, 128ci+c_local]; head of c_local<64
            # is 2ci, else 2ci+1, so the per-head diagonal slice is two
            # 64-partition-aligned block copies per (b, ci).
            wvT = big.tile([P, CH, M], f32, tag="wvT", name="wvT")
            for b in range(BPC):
                for ci in range(CH):
                    tp = pp.tile([P, P], f32, tag=f"t{ci % 2}", name=f"t{ci % 2}")
                    nc.tensor.transpose(
                        tp[:], On[b][:, ci * P:(ci + 1) * P], ident)
                    nc.vector.tensor_copy(
                        out=wvT[0:64, ci, b * T:(b + 1) * T],
                        in_=tp[0:64, 16 * ci:16 * ci + 8])
                    nc.vector.tensor_copy(
                        out=wvT[64:P, ci, b * T:(b + 1) * T],
                        in_=tp[64:P, 16 * ci + 8:16 * ci + 16])

            ps_fin = [pp.tile([M, 512], f32, tag=f"s{j}", name=f"s{j}") for j in range(2)]
            for ci in range(CH):
                wo = wpool.tile([P, C], f32, tag="wo", name="wo")
                nc.scalar.dma_start(wo[:], WoT[ci])
                for j in range(2):
                    nc.tensor.matmul(
                        ps_fin[j][:], wvT[:, ci, :], wo[:, j * 512:(j + 1) * 512],
                        start=(ci == 0), stop=(ci == CH - 1),
                    )
            fin = big.tile([M, C], f32, tag="fin", name="fin")
            for j in range(2):
                nc.scalar.copy(fin[:, j * 512:(j + 1) * 512], ps_fin[j][:])
            nc.vector.tensor_add(out=fin[:], in0=fin[:], in1=bob_sb[:])
            nc.sync.dma_start(out_d[:], fin[:])

    nc.compile()
    return nc


def _prep_host(x, kv_cache, Wq, bq, Wk, Wv, bv, Wo, bo, attn_dt_name):
    np_adt = {"bf16": ml_dtypes.bfloat16, "f32": np.float32}[attn_dt_name]
    f32 = np.float32
    x = np.asarray(x, f32)
    kv = np.asarray(kv_cache)
    Wq = np.asarray(Wq, f32); bq = np.asarray(bq, f32)
    Wk = np.asarray(Wk, f32); Wv = np.asarray(Wv, f32); bv = np.asarray(bv, f32)
    Wo = np.asarray(Wo, f32); bo = np.asarray(bo, f32)

    # K-cache transposed on host -> all device loads contiguous
    KT_all = np.ascontiguousarray(
        np.asarray(kv[:, 1, 0], f32).transpose(0, 2, 1)).astype(np_adt)  # [B, C, L]
    V_all = np.ascontiguousarray(np.asarray(kv[:, 1, 1], f32)).astype(np_adt)

    Wqkv = np.ascontiguousarray(
        np.concatenate([Wq.T, Wk.T, Wv.T], axis=1)).reshape(CH, P, 3 * C)
    WoT8 = np.ascontiguousarray(Wo.T).reshape(CH, P, C)
    bqs = np.ascontiguousarray((bq * SCALE).reshape(CH, P).T)  # [P, CH]
    bvb = np.ascontiguousarray(np.tile(bv, (M, 1)))
    bob = np.ascontiguousarray(np.tile(bo, (M, 1)))

    in_maps = []
    for c in range(NCORES):
        xc = x[c * BPC:(c + 1) * BPC].reshape(M, C)
        xTr = np.ascontiguousarray(xc.reshape(M, CH, P).transpose(2, 1, 0))
        in_maps.append({
            "xTr": xTr,
            "KT": np.ascontiguousarray(
                KT_all[c * BPC:(c + 1) * BPC]).reshape(BPC, CH, P, L),
            "Vd": np.ascontiguousarray(
                V_all[c * BPC:(c + 1) * BPC]).reshape(BPC, L // P, P, C),
            "Wqkv": Wqkv, "WoT": WoT8,
            "bqs": bqs, "bvb": bvb, "bob": bob,
        })
    return in_maps


def kernel(x, kv_cache, Wq, bq, Wk, Wv, bv, Wo, bo, _trace=False, _tmpdir=None):
    from concourse.bass_utils import run_bass_kernel_spmd

    _ensure_ntff_hook()
    key = ATTN_DT
    if key not in _CACHE:
        _CACHE[key] = _build(key)
    nc = _CACHE[key]

    in_maps = _prep_host(x, kv_cache, Wq, bq, Wk, Wv, bv, Wo, bo, key)
    res = run_bass_kernel_spmd(
        nc, in_maps, core_ids=list(range(NCORES)),
        trace=_trace, tmpdir=_tmpdir,
    )
    out = np.empty((B, T, C), np.float32)
    key_o = np.empty((B, T, C), np.float32)
    val_o = np.empty((B, T, C), np.float32)
    for c in range(NCORES):
        r = res.results[c]
        sl = slice(c * BPC, (c + 1) * BPC)
        out[sl] = r["out"].reshape(BPC, T, C)
        key_o[sl] = r["key"].reshape(BPC, T, C)
        val_o[sl] = r["value"].reshape(BPC, T, C)
    kernel._last_exec_time_ns = res.exec_time_ns
    kernel._last_results = res
    return (out, key_o, val_o)



# revision 6
# speedup vs baseline: 1.3890x; 1.3890x over previous
"""Trainium2 Bass kernel for cached multi-head self-attention decode step.

Problem (hardcoded):
  B=16, T=8, C=1024, n_head=16, head_dim=64, Lcache=4096, layer index 1.
  reference:
    q = x@Wq.T + bq ; key = x@Wk.T ; value = x@Wv.T + bv
    K = concat(kv_cache[:,1,0], key) ; V = concat(kv_cache[:,1,1], value)
    out = softmax((q*s)(K*s)^T) @ V @ Wo.T + bo      (s = hd**-0.25)
    returns (out, key, value)

Sharding: data-parallel over batch. 8 cores x 2 batches each. No collectives.

Per-core device algorithm (per batch b):
  - projections: q/k/v = xT.T @ [WqT|WkT|WvT], bf16 operands, fp32 PSUM.
  - PE-transpose q,k -> qT,kT; qT quantized to fp8 (scores ~N(0,1): safe).
  - QbigT[c-chunk, (h,m)]: block-diagonal fp8 packing of qT so one
    [128 x L] matmul chain computes all 16 heads x 8 queries of scores.
  - scores: fp8 DoubleRow matmuls (2 c-chunks per pass, 256-deep
    contraction) against the fp8 K-cache streamed in window-pair tiles.
  - softmax: exp(s - 3) fused into PSUM->SBUF copy (shift keeps the fp8
    weight range < 240), row sums via activation accum_out, normalization
    deferred to the attention output.
  - S@V: W-chunks PE-transposed to fp8 pairs, V streamed fp8 DoubleRow,
    PSUM-accumulated over 16 pair-chunks + one padded tail chunk
    (new kv rows).
  - out = (gathered wv)^T chunks @ WoT(bf16) + bo.

DMA plan: KT window-pairs on the sync queue (8 KB/partition contiguous
lines), V fully SBUF-resident via prologue pushes on the scalar queue,
weights on the gpsimd queue, outputs on the vector queue. All layouts are
pre-packed on the host so every device DMA is contiguous per partition.
"""

import sys
import types

import numpy as np
import ml_dtypes

# ---- hardcoded problem geometry ----
B, T, C = 16, 8, 1024
H, HD = 16, 64
L = 4096            # cached length
LT = L + T          # total keys
NCORES = 8
BPC = B // NCORES   # batches per core = 2
M = BPC * T         # queries per core = 16
P = 128
CH = C // P         # 8 c-chunks
NWG = 4             # window-groups (2 x 512 windows each)
NT = L // P         # 32 cached l-chunks
NPAIR = NT // 2     # 16 l-chunk pairs for S@V
SCALE = float(HD) ** -0.5  # applied once on q
ESHIFT = 3.0        # exp(s - ESHIFT): keeps fp8 softmax weights < 240

_CACHE = {}


def _ensure_ntff_hook():
    """run_bass_kernel_spmd(trace=True) under axon needs antenv.axon_hooks;
    shim it from the boot module if the image's antenv lacks it."""
    try:
        import antenv.axon_hooks  # noqa: F401
        return
    except ImportError:
        pass
    try:
        import trn_agent_boot.trn_boot as tb
        hook = tb._ntff_profile_via_ctypes("/opt/axon/libaxon_pjrt.so")
    except Exception:
        hook = None
    mod = types.ModuleType("antenv.axon_hooks")
    mod.get_axon_ntff_profile_hook = lambda: hook
    mod.set_axon_ntff_profile_hook = lambda h: None
    sys.modules["antenv.axon_hooks"] = mod


def _build():
    import concourse.bacc as bacc
    import concourse.mybir as mybir
    import concourse.tile as tile
    from concourse.masks import make_identity

    f32 = mybir.dt.float32
    bf16 = mybir.dt.bfloat16
    fp8 = mybir.dt.float8e4
    DR = mybir.MatmulPerfMode.DoubleRow

    nc = bacc.Bacc(None, target_bir_lowering=False)

    # ---- dram I/O ----
    xTr = nc.dram_tensor("xTr", [P, CH, M], bf16, kind="ExternalInput")
    # K cache, fp8, window-pair-major: [b, wg, pi, ch, lw2, 512]
    KT8 = nc.dram_tensor("KT8", [BPC, NWG, P, CH, 2, 512], fp8,
                         kind="ExternalInput")
    # V cache, fp8, chunk-major: [b, ts, pi, tt, c]
    V8 = nc.dram_tensor("V8", [BPC, 8, P, 4, C], fp8, kind="ExternalInput")
    Wqkv = nc.dram_tensor("Wqkv", [CH, P, 3 * C], bf16, kind="ExternalInput")
    WoT = nc.dram_tensor("WoT", [CH, P, C], bf16, kind="ExternalInput")
    bqs = nc.dram_tensor("bqs", [P, CH], f32, kind="ExternalInput")
    bvb = nc.dram_tensor("bvb", [M, C], f32, kind="ExternalInput")
    bob = nc.dram_tensor("bob", [M, C], f32, kind="ExternalInput")
    out_d = nc.dram_tensor("out", [M, C], f32, kind="ExternalOutput")
    key_d = nc.dram_tensor("key", [M, C], f32, kind="ExternalOutput")
    val_d = nc.dram_tensor("value", [M, C], f32, kind="ExternalOutput")

    AF = mybir.ActivationFunctionType
    AX = mybir.AxisListType
    OP = mybir.AluOpType

    with tile.TileContext(nc) as tc:
        with (
            tc.tile_pool(name="const", bufs=1) as cpool,
            tc.tile_pool(name="weights", bufs=2) as wpool,
            tc.tile_pool(name="kt", bufs=3) as ktpool,
            tc.tile_pool(name="wchunk", bufs=6) as wtpool,
            tc.tile_pool(name="big", bufs=1) as big,
            tc.tile_pool(name="ps", bufs=1, space="PSUM") as pp,
        ):
            # ---- constants + small loads (sync queue) ----
            ident = cpool.tile([P, P], f32, tag="ident", name="ident")
            make_identity(nc, ident)
            ident_b = cpool.tile([P, P], bf16, tag="ident_b", name="ident_b")
            nc.vector.tensor_copy(out=ident_b[:], in_=ident[:])
            xT_sb = cpool.tile([P, CH, M], bf16, tag="xT", name="xT")
            nc.sync.dma_start(xT_sb[:], xTr[:])
            bqs_sb = cpool.tile([P, CH], f32, tag="bqs", name="bqs")
            nc.sync.dma_start(bqs_sb[:], bqs[:])
            bvb_sb = cpool.tile([M, C], f32, tag="bvb", name="bvb")
            nc.sync.dma_start(bvb_sb[:], bvb[:])
            bob_sb = cpool.tile([M, C], f32, tag="bob", name="bob")
            nc.sync.dma_start(bob_sb[:], bob[:])
            nshift = cpool.tile([P, 1], f32, tag="nshift", name="nshift")
            nc.vector.memset(nshift[:], -ESHIFT)

            # ---- prologue: V cache fully resident, pushed on scalar ----
            vres = big.tile([P, BPC, 8, 4, C], fp8, tag="vres", name="vres")
            for b in range(BPC):
                for ts in range(8):
                    nc.scalar.dma_start(vres[:, b, ts], V8[b, ts])

            # ---- stage A: projections q/k/v (bf16 x f32-accum) ----
            ps_proj = [pp.tile([M, 512], f32, tag=t, name=f"proj_{t}")
                       for t in ("o0", "o1", "o2", "o3", "s0", "s1")]
            for ci in range(CH):
                wt = wpool.tile([P, 3 * C], bf16, tag="wqkv", name="wqkv")
                nc.gpsimd.dma_start(wt[:], Wqkv[ci])
                for j, ps in enumerate(ps_proj):
                    nc.tensor.matmul(
                        ps[:], xT_sb[:, ci, :], wt[:, j * 512:(j + 1) * 512],
                        start=(ci == 0), stop=(ci == CH - 1),
                    )
            # out-projection weights resident, also on gpsimd queue
            wores = big.tile([P, CH, C], bf16, tag="wores", name="wores")
            for ci in range(CH):
                nc.gpsimd.dma_start(wores[:, ci], WoT[ci])

            q_nat = big.tile([P, C], f32, tag="q_nat", name="q_nat")
            k_nat = big.tile([P, C], f32, tag="k_nat", name="k_nat")
            v_nat = big.tile([P, C], f32, tag="v_nat", name="v_nat")
            for t in (q_nat, k_nat, v_nat):
                nc.vector.memset(t[:], 0.0)
            for j in range(2):
                sl = slice(j * 512, (j + 1) * 512)
                nc.scalar.copy(q_nat[0:M, sl], ps_proj[0 + j][:])
                nc.scalar.copy(k_nat[0:M, sl], ps_proj[2 + j][:])
                nc.scalar.copy(v_nat[0:M, sl], ps_proj[4 + j][:])
            nc.vector.tensor_add(out=v_nat[0:M, :], in0=v_nat[0:M, :], in1=bvb_sb[:])
            nc.gpsimd.dma_start(key_d[:], k_nat[0:M, :])
            nc.gpsimd.dma_start(val_d[:], v_nat[0:M, :])

            # ---- qT / kT via PE transpose (fp8 outputs) ----
            qT = big.tile([P, CH, M], fp8, tag="qT", name="qT")
            kT = big.tile([P, CH, M], fp8, tag="kT", name="kT")
            for ci in range(CH):
                sl = slice(ci * P, (ci + 1) * P)
                tp = pp.tile([P, P], f32, tag="t0", name="t0")
                nc.tensor.transpose(tp[:], q_nat[:, sl], ident)
                # qT = (q + bq) * scale  (bqs host-prescaled by SCALE)
                nc.scalar.activation(
                    qT[:, ci, :], tp[:, 0:M], AF.Identity,
                    bias=bqs_sb[:, ci:ci + 1], scale=SCALE,
                )
                tp2 = pp.tile([P, P], f32, tag="t1", name="t1")
                nc.tensor.transpose(tp2[:], k_nat[:, sl], ident)
                nc.vector.tensor_copy(out=kT[:, ci, :], in_=tp2[:, 0:M])

            # ---- attention prep: block-diagonal fp8 Qbig per batch ----
            Qb, W_s, sums, rsum, ops_b, On = {}, {}, {}, {}, {}, {}
            for b in range(BPC):
                Qb[b] = big.tile([P, CH, P], fp8, tag=f"Qbig{b}", name=f"Qbig{b}")
                nc.vector.memset(Qb[b][:], 0.0)
                for ci in range(CH):
                    for j in range(2):
                        rows = slice(64 * j, 64 * (j + 1))
                        col0 = 16 * ci + 8 * j
                        nc.vector.tensor_copy(
                            out=Qb[b][rows, ci, col0:col0 + 8],
                            in_=qT[rows, ci, b * T:(b + 1) * T],
                        )
                W_s[b] = big.tile([P, LT], bf16, tag=f"W{b}", name=f"W{b}")
                sums[b] = big.tile([P, 16], f32, tag=f"sums{b}", name=f"sums{b}")

            # ---- scores: fp8 DoubleRow, window-pair KT tiles on sync ----
            # psum tags: b=0 -> s0/s1, b=1 -> t0/t1 (2 windows in flight each)
            sc_tags = {0: ("s0", "s1"), 1: ("t0", "t1")}
            for wg in range(NWG):
                for b in range(BPC):
                    kt = ktpool.tile([P, CH, 2, 512], fp8, tag="kt", name="kt")
                    nc.sync.dma_start(kt[:], KT8[b, wg])
                    sps = [pp.tile([P, 512], f32, tag=sc_tags[b][u],
                                   name=f"sp{b}{u}") for u in range(2)]
                    for t in range(4):
                        for u in range(2):
                            nc.tensor.matmul(
                                sps[u][:], Qb[b][:, 2 * t:2 * t + 2, :],
                                kt[:, 2 * t:2 * t + 2, u, :],
                                start=(t == 0), stop=(t == 3),
                                perf_mode=DR,
                            )
                    for u in range(2):
                        lw = 2 * wg + u
                        nc.scalar.activation(
                            W_s[b][:, lw * 512:(lw + 1) * 512], sps[u][:],
                            AF.Exp, bias=nshift[:],
                            accum_out=sums[b][:, lw:lw + 1],
                        )
            for b in range(BPC):
                # scores against the T new keys (plain fp8 matmul)
                spn = pp.tile([P, 512], f32, tag=sc_tags[b][0], name="spn")
                for ci in range(CH):
                    nc.tensor.matmul(
                        spn[:, 0:T], Qb[b][:, ci, :], kT[:, ci, b * T:(b + 1) * T],
                        start=(ci == 0), stop=(ci == CH - 1),
                    )
                nc.scalar.activation(
                    W_s[b][:, L:LT], spn[:, 0:T], AF.Exp, bias=nshift[:],
                    accum_out=sums[b][:, 8:9],
                )
                rs = big.tile([P, 1], f32, tag=f"rs{b}", name=f"rs{b}")
                nc.vector.tensor_reduce(
                    out=rs[:], in_=sums[b][:, 0:9], axis=AX.X, op=OP.add)
                rsum[b] = big.tile([P, 1], f32, tag=f"rsum{b}", name=f"rsum{b}")
                nc.vector.reciprocal(rsum[b][:], rs[:])

                # padded tail operands (new kv rows), all fp8.
                if b == 0:
                    vpad = big.tile([P, C], fp8, tag="vpad", name="vpad")
                    nc.vector.memset(vpad[:], 0.0)
                    nc.vector.tensor_copy(out=vpad[0:M, :], in_=v_nat[0:M, :])
                wt32 = big.tile([P, P], fp8, tag=f"wt32_{b}", name=f"wt32_{b}")
                nc.vector.memset(wt32[:], 0.0)
                wn_pad = big.tile([P, M], bf16, tag=f"wn_pad{b}", name=f"wn_pad{b}")
                nc.vector.memset(wn_pad[:], 0.0)
                nc.vector.tensor_copy(
                    out=wn_pad[:, b * T:(b + 1) * T], in_=W_s[b][:, L:LT])
                tpn = pp.tile([P, P], bf16, tag=sc_tags[b][1], name="tpn")
                nc.tensor.transpose(tpn[0:M, :], wn_pad[:], ident_b)
                nc.vector.tensor_copy(out=wt32[0:M, :], in_=tpn[0:M, :])
                ops_b[b] = (wt32,
                            [pp.tile([P, 512], f32, tag=f"o{2 * b + j}",
                                     name=f"o{2 * b + j}") for j in range(2)])

            # ---- S@V: fp8 DoubleRow over 16 l-chunk pairs + padded tail ----
            # W chunks PE-transposed (bf16) then packed as fp8 pairs.
            for tp_ in range(NPAIR):
                for b in range(BPC):
                    ops = ops_b[b][1]
                    wts = wtpool.tile([P, 2, P], fp8, tag="wt", name="wt")
                    for u in range(2):
                        t_ = 2 * tp_ + u
                        tpw = pp.tile([P, P], bf16, tag=f"t{(2 * t_ + b) % 2}",
                                      name="tpw")
                        nc.tensor.transpose(
                            tpw[:], W_s[b][:, t_ * P:(t_ + 1) * P], ident_b)
                        nc.vector.tensor_copy(out=wts[:, u, :], in_=tpw[:])
                    ts, jp = divmod(tp_, 2)
                    for j in range(2):
                        nc.tensor.matmul(
                            ops[j][:], wts[:],
                            vres[:, b, ts, 2 * jp:2 * jp + 2,
                                 j * 512:(j + 1) * 512],
                            start=(tp_ == 0), stop=False,
                            perf_mode=DR,
                        )
            for b in range(BPC):
                wt32, ops = ops_b[b]
                for j in range(2):
                    nc.tensor.matmul(
                        ops[j][:], wt32[:], vpad[:, j * 512:(j + 1) * 512],
                        start=False, stop=True,
                    )
                # normalize rows while copying out of PSUM
                On[b] = big.tile([P, C], f32, tag=f"On{b}", name=f"On{b}")
                for j in range(2):
                    nc.scalar.activation(
                        On[b][:, j * 512:(j + 1) * 512], ops[j][:], AF.Copy,
                        scale=rsum[b][:],
                    )

            # ---- wvT directly from transposed On chunks ----
            # OnT[c_local, (h,t)] = On[8h+t, 128ci+c_local]; head of c_local<64
            # is 2ci, else 2ci+1, so the per-head diagonal slice is two
            # 64-partition-aligned block copies per (b, ci).
            wvT = big.tile([P, CH, M], bf16, tag="wvT", name="wvT")
            for b in range(BPC):
                for ci in range(CH):
                    tp = pp.tile([P, P], f32, tag=f"t{ci % 2}", name=f"t{ci % 2}")
                    nc.tensor.transpose(
                        tp[:], On[b][:, ci * P:(ci + 1) * P], ident)
                    nc.vector.tensor_copy(
                        out=wvT[0:64, ci, b * T:(b + 1) * T],
                        in_=tp[0:64, 16 * ci:16 * ci + 8])
                    nc.vector.tensor_copy(
                        out=wvT[64:P, ci, b * T:(b + 1) * T],
                        in_=tp[64:P, 16 * ci + 8:16 * ci + 16])

            ps_fin = [pp.tile([M, 512], f32, tag=f"s{j}", name=f"fin{j}")
                      for j in range(2)]
            for ci in range(CH):
                for j in range(2):
                    nc.tensor.matmul(
                        ps_fin[j][:], wvT[:, ci, :],
                        wores[:, ci, j * 512:(j + 1) * 512],
                        start=(ci == 0), stop=(ci == CH - 1),
                    )
            fin = big.tile([M, C], f32, tag="fin", name="fin")
            for j in range(2):
                nc.scalar.copy(fin[:, j * 512:(j + 1) * 512], ps_fin[j][:])
            nc.vector.tensor_add(out=fin[:], in0=fin[:], in1=bob_sb[:])
            nc.gpsimd.dma_start(out_d[:], fin[:])

    nc.compile()
    return nc


def _prep_host(x, kv_cache, Wq, bq, Wk, Wv, bv, Wo, bo):
    bf16 = ml_dtypes.bfloat16
    fp8 = ml_dtypes.float8_e4m3fn
    f32 = np.float32
    x = np.asarray(x, f32)
    kv = np.asarray(kv_cache)
    Wq = np.asarray(Wq, f32); bq = np.asarray(bq, f32)
    Wk = np.asarray(Wk, f32); Wv = np.asarray(Wv, f32); bv = np.asarray(bv, f32)
    Wo = np.asarray(Wo, f32); bo = np.asarray(bo, f32)

    K_all = np.asarray(kv[:, 1, 0], f32)       # [B, L, C]
    V_all = np.asarray(kv[:, 1, 1], f32)       # [B, L, C]
    # window-pair-major fp8 KT: [B, wg, pi, ch, lw2, 512]
    KT8_all = np.ascontiguousarray(
        K_all.reshape(B, NWG, 2, 512, CH, P).transpose(0, 1, 5, 4, 2, 3)
    ).astype(fp8)
    # chunk-major fp8 V: [B, ts, pi, tt, c]
    V8_all = np.ascontiguousarray(
        V_all.reshape(B, 8, 4, P, C).transpose(0, 1, 3, 2, 4)
    ).astype(fp8)

    Wqkv = np.ascontiguousarray(
        np.concatenate([Wq.T, Wk.T, Wv.T], axis=1)).reshape(CH, P, 3 * C)
    Wqkv = Wqkv.astype(bf16)
    WoT8 = np.ascontiguousarray(Wo.T).reshape(CH, P, C).astype(bf16)
    bqs = np.ascontiguousarray((bq * SCALE).reshape(CH, P).T)  # [P, CH]
    bvb = np.ascontiguousarray(np.tile(bv, (M, 1)))
    bob = np.ascontiguousarray(np.tile(bo, (M, 1)))

    in_maps = []
    for c in range(NCORES):
        xc = x[c * BPC:(c + 1) * BPC].reshape(M, C)
        xTr = np.ascontiguousarray(
            xc.reshape(M, CH, P).transpose(2, 1, 0)).astype(bf16)
        in_maps.append({
            "xTr": xTr,
            "KT8": np.ascontiguousarray(KT8_all[c * BPC:(c + 1) * BPC]),
            "V8": np.ascontiguousarray(V8_all[c * BPC:(c + 1) * BPC]),
            "Wqkv": Wqkv, "WoT": WoT8,
            "bqs": bqs, "bvb": bvb, "bob": bob,
        })
    return in_maps


def kernel(x, kv_cache, Wq, bq, Wk, Wv, bv, Wo, bo, _trace=False, _tmpdir=None):
    from concourse.bass_utils import run_bass_kernel_spmd

    _ensure_ntff_hook()
    if "nc" not in _CACHE:
        _CACHE["nc"] = _build()
    nc = _CACHE["nc"]

    in_maps = _prep_host(x, kv_cache, Wq, bq, Wk, Wv, bv, Wo, bo)
    res = run_bass_kernel_spmd(
        nc, in_maps, core_ids=list(range(NCORES)),
        trace=_trace, tmpdir=_tmpdir,
    )
    out = np.empty((B, T, C), np.float32)
    key_o = np.empty((B, T, C), np.float32)
    val_o = np.empty((B, T, C), np.float32)
    for c in range(NCORES):
        r = res.results[c]
        sl = slice(c * BPC, (c + 1) * BPC)
        out[sl] = r["out"].reshape(BPC, T, C)
        key_o[sl] = r["key"].reshape(BPC, T, C)
        val_o[sl] = r["value"].reshape(BPC, T, C)
    kernel._last_exec_time_ns = res.exec_time_ns
    kernel._last_results = res
    return (out, key_o, val_o)
